# revision 24
# baseline (speedup 1.0000x reference)
"""Trainium2 Bass kernel for nn_Chord_decoder (attention + LSTM cell + vocab head).

Distribution over 8 NeuronCores (tensor-parallel, per sharding hint):
  - encoder_outputs sharded over L (512 rows/core, both layouts) with a
    softmax-stats AllGather to combine attention partials,
  - W_ih/W_hh sharded over the 4H gate dim (gate-interleaved so core k
    owns h/c slice [128k:128k+128]), h_new recombined with an AllGather,
  - out_W/out_b/emb sharded over the vocab dim; the big out_W GEMV runs
    on the Vector engine (fused multiply+reduce) against natural-layout
    weight tiles streamed from HBM through a ring buffer — the dominant
    memory stream, overlapped with everything before it.

Raw-Bass SPMD program (one Block, manual semaphores): the Tile layer's
attached-wait encoding is not supported by this walrus build, so the
kernel is written against explicit per-engine streams.

kernel(**inputs) accepts the FULL unsharded inputs and returns the full
(logits, (h_n, c_n)) structure, matching the reference.
"""

import sys
from contextlib import ExitStack

for _p in ("/root/.axon_site/_ro/trn_rl_repo", "/opt/trn_rl_repo"):
    if _p not in sys.path:
        sys.path.append(_p)

import numpy as np

import concourse.bass as bass
import concourse.mybir as mybir
from concourse.bass import ts
from concourse import bass_utils

F32 = mybir.dt.float32
U32 = mybir.dt.uint32
AF = mybir.ActivationFunctionType
ALU = mybir.AluOpType
AX = mybir.AxisListType

NCORES = 8
P = 128
H, E, V, L = 1024, 512, 50257, 4096
LS = L // NCORES          # 512  L rows per core
GS = 4 * H // NCORES      # 512  gate rows per core (4 gates x 128)
HC = H // P               # 8    128-chunks of H
VS = 6400                 # vocab rows per core (padded: 8*6400 = 51200)
VT = VS // P              # 50   vocab tiles per core
ES = -(-V // NCORES)      # 6283 emb rows per core (padded: 8*6283 = 50264)
AG1 = 2 + H + E           # [m, Z, u(1024), x_emb(512)] = 1538
RB = 21                   # out_W ring-buffer depth (x 0.5 MB)

# semaphore milestone values
DS_ALL = 12 * 16
P_SCORES, P_PT, P_U, P_ST, P_WT, P_CTX, P_CC, P_PROJ, P_XC, P_GATES, P_LT = range(1, 12)
(V_ONES, V_NEGM, V_PCOLS, V_AG1, V_NEGMG, V_WROW, V_W8, V_CTXSB, V_CTXCOLS,
 V_XY, V_XCOLS, V_GATESSB, V_CNEW, V_HNEW, V_LFIN) = range(1, 16)
C_EXP, C_EROW, C_ACTS, C_TANH = 1, 2, 6, 7
M_AG1IN, M_AGSB, M_AG2IN, M_HREP = 16, 32, 48, 64
G_AG1, G_AG2 = 1, 2


def build_nc():
    nc = bass.Bass(num_devices=NCORES)

    # ---- per-core inputs (host pre-sharded / pre-transposed) ----
    encT = nc.dram_tensor("encT", [H, LS], F32, kind="ExternalInput")
    enc_nat = nc.dram_tensor("enc_nat", [LS, H], F32, kind="ExternalInput")
    h_cols = nc.dram_tensor("h_cols", [P, HC], F32, kind="ExternalInput")
    c_sh = nc.dram_tensor("c_sh", [1, P], F32, kind="ExternalInput")
    w_combT = nc.dram_tensor("w_combT", [2 * H, GS], F32, kind="ExternalInput")
    b_comb = nc.dram_tensor("b_comb", [1, GS], F32, kind="ExternalInput")
    attn_wT = nc.dram_tensor("attn_wT", [H, E], F32, kind="ExternalInput")
    attn_b_t = nc.dram_tensor("attn_b_t", [1, E], F32, kind="ExternalInput")
    emb_sh = nc.dram_tensor("emb_sh", [ES, E], F32, kind="ExternalInput")
    tok_off = nc.dram_tensor("tok_off", [2, 1], U32, kind="ExternalInput")
    out_w_nat = nc.dram_tensor("out_w_nat", [VS, H], F32, kind="ExternalInput")
    out_b_cols = nc.dram_tensor("out_b_cols", [P, VT], F32, kind="ExternalInput")

    # ---- per-core outputs ----
    logits_sh = nc.dram_tensor("logits_sh", [VT, P], F32, kind="ExternalOutput")
    h_new_sh = nc.dram_tensor("h_new_sh", [1, P], F32, kind="ExternalOutput")
    c_new_sh = nc.dram_tensor("c_new_sh", [1, P], F32, kind="ExternalOutput")

    # ---- NEFF-embedded constants ----
    eye8_d = nc.inline_tensor(np.eye(8, dtype=np.float32), "eye8")
    eye128_d = nc.inline_tensor(np.eye(128, dtype=np.float32), "eye128")

    # ---- internal DRAM (collective bounce buffers) ----
    ag1_in = nc.dram_tensor("ag1_in", [1, AG1], F32)
    ag1_out = nc.dram_tensor("ag1_out", [NCORES, AG1], F32)
    ag2_in = nc.dram_tensor("ag2_in", [1, P], F32)
    ag2_out = nc.dram_tensor("ag2_out", [NCORES, P], F32)

    groups = [list(range(NCORES))]

    with ExitStack() as st:
        sb = lambda name, shape, dt=F32: st.enter_context(
            nc.sbuf_tensor(name, shape, dt)
        )
        # big residents
        encT_sb = sb("encT_sb", [P, HC, LS])
        enc_nat_sb = sb("enc_nat_sb", [P, LS // P, H])
        attn_wT_sb = sb("attn_wT_sb", [P, HC, E])
        w_combT_sb = sb("w_combT_sb", [P, 2 * HC, GS])
        w_ring = sb("w_ring", [P, RB, H])
        h_rep = sb("h_rep", [P, H])
        # small tensors
        h_cols_sb = sb("h_cols_sb", [P, HC])
        c_sb = sb("c_sb", [1, P])
        b_comb_sb = sb("b_comb_sb", [1, GS])
        attn_b_sb = sb("attn_b_sb", [1, E])
        out_b_sb = sb("out_b_sb", [P, VT])
        eye8_sb = sb("eye8_sb", [8, 8])
        eye128_sb = sb("eye128_sb", [P, P])
        tok_sb = sb("tok_sb", [2, 1], U32)
        ones1 = sb("ones1", [1, 1])
        ones8 = sb("ones8", [8, 1])
        m_loc = sb("m_loc", [1, 1])
        neg_m = sb("neg_m", [1, 1])
        p_vec = sb("p_vec", [1, LS])
        z_loc = sb("z_loc", [1, 1])
        p_cols = sb("p_cols", [P, LS // P])
        xe_sb = sb("xe_sb", [2, E])
        ag1_sb = sb("ag1_sb", [1, AG1])
        ag_sb = sb("ag_sb", [NCORES, AG1])
        m_glob = sb("m_glob", [1, 1])
        neg_mg = sb("neg_mg", [1, 1])
        e_row = sb("e_row", [1, 8])
        zsc = sb("zsc", [1, 8])
        z_glob = sb("z_glob", [1, 1])
        z_inv = sb("z_inv", [1, 1])
        w_row = sb("w_row", [1, 8])
        w8_sb = sb("w8_sb", [8, 1])
        ctx_sb = sb("ctx_sb", [1, H])
        ctx_cols = sb("ctx_cols", [P, HC])
        xy_sb = sb("xy_sb", [1, 2 * E])
        x_cols = sb("x_cols", [P, HC])
        gates_sb = sb("gates_sb", [1, GS])
        i_s = sb("i_s", [1, P])
        f_s = sb("f_s", [1, P])
        g_t = sb("g_t", [1, P])
        o_s = sb("o_s", [1, P])
        fc = sb("fc", [1, P])
        ig = sb("ig", [1, P])
        c_new = sb("c_new", [1, P])
        tc_sb = sb("tc_sb", [1, P])
        h_new = sb("h_new", [1, P])
        lacc = sb("lacc", [P, VT])
        lbias = sb("lbias", [P, VT])
        lfin = sb("lfin", [VT, P])

        ps = st.enter_context(nc.psum_tensor("ps", [P, 8, 512], F32))

        sem = lambda name: st.enter_context(nc.semaphore(name))
        dS = sem("dS")    # setup DMAs
        sP = sem("sP")    # PE milestones
        sV = sem("sV")    # DVE milestones
        sC = sem("sC")    # ACT milestones
        sM = sem("sM")    # mid-phase DMAs (ACT-issued)
        sG = sem("sG")    # collectives
        sX = sem("sX")    # emb gather DMA
        sXm = sem("sXm")  # xe memset ordering
        sL = sem("sL")    # per-TTR logits progress
        sO = sem("sO")    # output DMAs
        wsem = [sem(f"wsl{j}") for j in range(RB)]   # per-ring-slot DMA sems

        with nc.Block() as block:

            @block.sync
            def _(sync):
                # setup DMAs (12 x then_inc(dS, 16))
                sync.dma_start(
                    out=encT_sb[:, :, :],
                    in_=encT[:, :].rearrange("(c p) l -> p c l", p=P),
                ).then_inc(dS, 16)
                sync.dma_start(
                    out=enc_nat_sb[:, :, :],
                    in_=enc_nat[:, :].rearrange("(c p) h -> p c h", p=P),
                ).then_inc(dS, 16)
                sync.dma_start(
                    out=attn_wT_sb[:, :, :],
                    in_=attn_wT[:, :].rearrange("(c p) e -> p c e", p=P),
                ).then_inc(dS, 16)
                sync.dma_start(
                    out=w_combT_sb[:, :, :],
                    in_=w_combT[:, :].rearrange("(c p) g -> p c g", p=P),
                ).then_inc(dS, 16)
                sync.dma_start(out=h_cols_sb[:, :], in_=h_cols[:, :]).then_inc(dS, 16)
                sync.dma_start(out=c_sb[:, :], in_=c_sh[:, :]).then_inc(dS, 16)
                sync.dma_start(out=b_comb_sb[:, :], in_=b_comb[:, :]).then_inc(dS, 16)
                sync.dma_start(out=attn_b_sb[:, :], in_=attn_b_t[:, :]).then_inc(dS, 16)
                sync.dma_start(out=out_b_sb[:, :], in_=out_b_cols[:, :]).then_inc(dS, 16)
                sync.dma_start(out=eye8_sb[:, :], in_=eye8_d[:, :]).then_inc(dS, 16)
                sync.dma_start(out=eye128_sb[:, :], in_=eye128_d[:, :]).then_inc(dS, 16)
                sync.dma_start(out=tok_sb[:, :], in_=tok_off[:, :]).then_inc(dS, 16)
                # out_W stream through the ring
                for t in range(VT):
                    j = t % RB
                    if t >= RB:
                        # slot reuse: wait until the TTR for tile t-RB retired
                        sync.wait_ge(sL, t - RB + 1)
                    sync.dma_start(
                        out=w_ring[:, j, :], in_=out_w_nat[ts(t, P), :]
                    ).then_inc(wsem[j], 16)
                # final logits store
                sync.wait_ge(sV, V_LFIN)
                sync.dma_start(out=logits_sh[:, :], in_=lfin[:, :]).then_inc(sO, 16)
                sync.wait_ge(sO, 48)

            @block.gpsimd
            def _(gpsimd):
                gpsimd.memset(xe_sb[:, :], 0.0).then_inc(sXm, 1)
                gpsimd.wait_ge(dS, DS_ALL)
                gpsimd.wait_ge(sXm, 1)
                gpsimd.indirect_dma_start(
                    out=xe_sb[:, :],
                    out_offset=None,
                    in_=emb_sh[:, :],
                    in_offset=bass.IndirectOffsetOnAxis(ap=tok_sb[:, :1], axis=0),
                    bounds_check=ES - 1,
                    oob_is_err=False,
                ).then_inc(sX, 16)
                gpsimd.wait_ge(sM, M_AG1IN)
                gpsimd.collective_compute(
                    "AllGather",
                    ALU.bypass,
                    replica_groups=groups,
                    ins=[ag1_in[:, :].opt()],
                    outs=[ag1_out[:, :].opt()],
                ).then_inc(sG, 1)
                gpsimd.wait_ge(sM, M_AG2IN)
                gpsimd.collective_compute(
                    "AllGather",
                    ALU.bypass,
                    replica_groups=groups,
                    ins=[ag2_in[:, :].opt()],
                    outs=[ag2_out[:, :].opt()],
                ).then_inc(sG, 1)

            @block.tensor
            def _(tensor):
                tensor.wait_ge(dS, DS_ALL)
                # scores = enc @ h  -> ps[b0] [1, 512]
                for c in range(HC):
                    mm = tensor.matmul(
                        ps[0:1, 0, :],
                        lhsT=h_cols_sb[:, c : c + 1],
                        rhs=encT_sb[:, c, :],
                        start=(c == 0),
                        stop=(c == HC - 1),
                    )
                mm.then_inc(sP, 1)  # P_SCORES
                # p transposes -> ps[b1] [128, 4]
                tensor.wait_ge(sC, C_EXP)
                tensor.wait_ge(sV, V_ONES)
                for j in range(LS // P):
                    mm = tensor.matmul(
                        ps[0:P, 1, j : j + 1],
                        lhsT=p_vec[0:1, ts(j, P)],
                        rhs=ones1[0:1, 0:1],
                        start=True,
                        stop=True,
                    )
                mm.then_inc(sP, 1)  # P_PT
                # u = p @ enc -> ps[b2:b4] [1, 1024]
                tensor.wait_ge(sV, V_PCOLS)
                for c in range(LS // P):
                    for nb in range(2):
                        mm = tensor.matmul(
                            ps[0:1, 2 + nb, :],
                            lhsT=p_cols[:, c : c + 1],
                            rhs=enc_nat_sb[:, c, ts(nb, 512)],
                            start=(c == 0),
                            stop=(c == LS // P - 1),
                        )
                mm.then_inc(sP, 1)  # P_U
                # gathered stats transposes -> ps[b4] [1, 16]
                tensor.wait_ge(sM, M_AGSB)
                tensor.matmul(
                    ps[0:1, 4, 0:8], lhsT=ag_sb[:, 0:1], rhs=eye8_sb[:, :],
                    start=True, stop=True,
                )
                tensor.matmul(
                    ps[0:1, 4, 8:16], lhsT=ag_sb[:, 1:2], rhs=eye8_sb[:, :],
                    start=True, stop=True,
                ).then_inc(sP, 1)  # P_ST
                # w transpose -> ps[b5] [8, 1]
                tensor.wait_ge(sV, V_WROW)
                tensor.matmul(
                    ps[0:8, 5, 0:1], lhsT=w_row[0:1, :], rhs=ones1[0:1, 0:1],
                    start=True, stop=True,
                ).then_inc(sP, 1)  # P_WT
                # ctx combine -> ps[b0:b2] [1, 1024]; x_emb sum -> ps[b6] [1, 512]
                tensor.wait_ge(sV, V_W8)
                for nb in range(2):
                    tensor.matmul(
                        ps[0:1, nb, :],
                        lhsT=w8_sb[:, 0:1],
                        rhs=ag_sb[:, 2 + nb * 512 : 2 + (nb + 1) * 512],
                        start=True,
                        stop=True,
                    )
                tensor.matmul(
                    ps[0:1, 6, :], lhsT=ones8[:, 0:1], rhs=ag_sb[:, 2 + H : AG1],
                    start=True, stop=True,
                ).then_inc(sP, 1)  # P_CTX
                # ctx transposes -> ps[b7] [128, 8]
                tensor.wait_ge(sV, V_CTXSB)
                for j in range(HC):
                    mm = tensor.matmul(
                        ps[0:P, 7, j : j + 1],
                        lhsT=ctx_sb[0:1, ts(j, P)],
                        rhs=ones1[0:1, 0:1],
                        start=True,
                        stop=True,
                    )
                mm.then_inc(sP, 1)  # P_CC
                # attention projection -> ps[b4] [1, 512]
                tensor.wait_ge(sV, V_CTXCOLS)
                for c in range(HC):
                    mm = tensor.matmul(
                        ps[0:1, 4, :],
                        lhsT=ctx_cols[:, c : c + 1],
                        rhs=attn_wT_sb[:, c, :],
                        start=(c == 0),
                        stop=(c == HC - 1),
                    )
                mm.then_inc(sP, 1)  # P_PROJ
                # x transposes -> ps[b5] [128, 8]
                tensor.wait_ge(sV, V_XY)
                for j in range(HC):
                    mm = tensor.matmul(
                        ps[0:P, 5, j : j + 1],
                        lhsT=xy_sb[0:1, ts(j, P)],
                        rhs=ones1[0:1, 0:1],
                        start=True,
                        stop=True,
                    )
                mm.then_inc(sP, 1)  # P_XC
                # LSTM gates -> ps[b6] [1, 512]
                tensor.wait_ge(sV, V_XCOLS)
                for c in range(2 * HC):
                    lhs = (
                        x_cols[:, c : c + 1]
                        if c < HC
                        else h_cols_sb[:, c - HC : c - HC + 1]
                    )
                    mm = tensor.matmul(
                        ps[0:1, 6, :],
                        lhsT=lhs,
                        rhs=w_combT_sb[:, c, :],
                        start=(c == 0),
                        stop=(c == 2 * HC - 1),
                    )
                mm.then_inc(sP, 1)  # P_GATES
                # final logits transpose -> ps[b0] [50, 128]
                tensor.wait_ge(sL, VT + 1)
                tensor.transpose(
                    ps[0:VT, 0, 0:P], lbias[:, :], eye128_sb[:, :]
                ).then_inc(sP, 1)  # P_LT

            @block.scalar
            def _(scalar):
                # exp(scores - m) with fused sum
                scalar.wait_ge(sP, P_SCORES)
                scalar.wait_ge(sV, V_NEGM)
                scalar.activation(
                    p_vec[:, :], ps[0:1, 0, :], AF.Exp, bias=neg_m[0:1, :],
                    scale=1.0, accum_out=z_loc[:, :],
                ).then_inc(sC, 1)  # C_EXP
                # mid DMAs: ag1 out/in
                scalar.wait_ge(sV, V_AG1)
                scalar.dma_start(out=ag1_in[:, :], in_=ag1_sb[:, :]).then_inc(sM, 16)
                scalar.wait_ge(sG, G_AG1)
                scalar.dma_start(out=ag_sb[:, :], in_=ag1_out[:, :]).then_inc(sM, 16)
                # e_row = exp(m_p - M)
                scalar.wait_ge(sP, P_ST)
                scalar.wait_ge(sV, V_NEGMG)
                scalar.activation(
                    e_row[:, :], ps[0:1, 4, 0:8], AF.Exp, bias=neg_mg[0:1, :]
                ).then_inc(sC, 1)  # C_EROW
                # gate activations
                scalar.wait_ge(sV, V_GATESSB)
                scalar.activation(i_s[:, :], gates_sb[0:1, 0:P], AF.Sigmoid).then_inc(sC, 1)
                scalar.activation(f_s[:, :], gates_sb[0:1, P : 2 * P], AF.Sigmoid).then_inc(sC, 1)
                scalar.activation(g_t[:, :], gates_sb[0:1, 2 * P : 3 * P], AF.Tanh).then_inc(sC, 1)
                scalar.activation(o_s[:, :], gates_sb[0:1, 3 * P : 4 * P], AF.Sigmoid).then_inc(sC, 1)  # C_ACTS
                scalar.wait_ge(sV, V_CNEW)
                scalar.activation(tc_sb[:, :], c_new[0:1, :], AF.Tanh).then_inc(sC, 1)  # C_TANH
                # outputs + AG2 feed + h replicate
                scalar.wait_ge(sV, V_HNEW)
                scalar.dma_start(out=h_new_sh[:, :], in_=h_new[:, :]).then_inc(sO, 16)
                scalar.dma_start(out=c_new_sh[:, :], in_=c_new[:, :]).then_inc(sO, 16)
                scalar.dma_start(out=ag2_in[:, :], in_=h_new[:, :]).then_inc(sM, 16)
                scalar.wait_ge(sG, G_AG2)
                scalar.dma_start(
                    out=h_rep[:, :],
                    in_=ag2_out[:, :].rearrange("a b -> (a b)").partition_broadcast(P),
                ).then_inc(sM, 16)

            @block.vector
            def _(vector):
                vector.memset(ones1[:, :], 1.0)
                vector.memset(ones8[:, :], 1.0).then_inc(sV, 1)  # V_ONES
                # local softmax stats
                vector.wait_ge(sP, P_SCORES)
                vector.reduce_max(m_loc[:, :], ps[0:1, 0, :], axis=AX.X)
                vector.drain()
                vector.tensor_scalar_mul(neg_m[:, :], m_loc[:, :], -1.0).then_inc(sV, 1)  # V_NEGM
                vector.wait_ge(sP, P_PT)
                vector.tensor_copy(p_cols[:, :], ps[0:P, 1, 0:4]).then_inc(sV, 1)  # V_PCOLS
                # assemble AG1 payload
                vector.wait_ge(sP, P_U)
                vector.wait_ge(sC, C_EXP)
                vector.wait_ge(sX, 16)
                vector.tensor_copy(ag1_sb[0:1, 0:1], m_loc[:, :])
                vector.tensor_copy(ag1_sb[0:1, 1:2], z_loc[:, :])
                vector.tensor_copy(ag1_sb[0:1, 2 : 2 + 512], ps[0:1, 2, :])
                vector.tensor_copy(ag1_sb[0:1, 2 + 512 : 2 + H], ps[0:1, 3, :])
                vector.tensor_copy(ag1_sb[0:1, 2 + H : AG1], xe_sb[0:1, :]).then_inc(sV, 1)  # V_AG1
                # global softmax combine
                vector.wait_ge(sP, P_ST)
                vector.reduce_max(m_glob[:, :], ps[0:1, 4, 0:8], axis=AX.X)
                vector.drain()
                vector.tensor_scalar_mul(neg_mg[:, :], m_glob[:, :], -1.0).then_inc(sV, 1)  # V_NEGMG
                vector.wait_ge(sC, C_EROW)
                vector.tensor_tensor(zsc[:, :], e_row[:, :], ps[0:1, 4, 8:16], op=ALU.mult)
                vector.drain()
                vector.reduce_sum(z_glob[:, :], zsc[:, :], axis=AX.X)
                vector.drain()
                vector.reciprocal(z_inv[:, :], z_glob[:, :])
                vector.drain()
                vector.tensor_scalar_mul(w_row[:, :], e_row[:, :], z_inv[0:1, :]).then_inc(sV, 1)  # V_WROW
                vector.wait_ge(sP, P_WT)
                vector.tensor_copy(w8_sb[:, :], ps[0:8, 5, 0:1]).then_inc(sV, 1)  # V_W8
                vector.wait_ge(sP, P_CTX)
                vector.tensor_copy(ctx_sb[0:1, 0:512], ps[0:1, 0, :])
                vector.tensor_copy(ctx_sb[0:1, 512:H], ps[0:1, 1, :]).then_inc(sV, 1)  # V_CTXSB
                vector.wait_ge(sP, P_CC)
                vector.tensor_copy(ctx_cols[:, :], ps[0:P, 7, 0:8]).then_inc(sV, 1)  # V_CTXCOLS
                vector.wait_ge(sP, P_PROJ)
                vector.tensor_add(xy_sb[0:1, 0:E], ps[0:1, 4, :], attn_b_sb[:, :])
                vector.tensor_copy(xy_sb[0:1, E : 2 * E], ps[0:1, 6, :]).then_inc(sV, 1)  # V_XY
                vector.wait_ge(sP, P_XC)
                vector.tensor_copy(x_cols[:, :], ps[0:P, 5, 0:8]).then_inc(sV, 1)  # V_XCOLS
                vector.wait_ge(sP, P_GATES)
                vector.tensor_add(gates_sb[:, :], ps[0:1, 6, :], b_comb_sb[:, :]).then_inc(sV, 1)  # V_GATESSB
                # LSTM cell pointwise
                vector.wait_ge(sC, C_ACTS)
                vector.tensor_tensor(fc[:, :], f_s[:, :], c_sb[:, :], op=ALU.mult)
                vector.tensor_tensor(ig[:, :], i_s[:, :], g_t[:, :], op=ALU.mult)
                vector.drain()
                vector.tensor_tensor(c_new[:, :], fc[:, :], ig[:, :], op=ALU.add).then_inc(sV, 1)  # V_CNEW
                vector.wait_ge(sC, C_TANH)
                vector.tensor_tensor(h_new[:, :], o_s[:, :], tc_sb[:, :], op=ALU.mult).then_inc(sV, 1)  # V_HNEW
                # vocab head: fused multiply+reduce per 128-row out_W tile
                vector.wait_ge(sM, M_HREP)
                for t in range(VT):
                    j = t % RB
                    vector.wait_ge(wsem[j], 16 * (t // RB + 1))
                    # scratch product lands in retired PSUM banks (b2-3/b4-5)
                    pb = 2 + (t % 2) * 2
                    vector.scalar_tensor_tensor(
                        out=ps[0:P, pb : pb + 2, :].rearrange("p a b -> p (a b)"),
                        in0=w_ring[:, j, :],
                        scalar=1.0,
                        in1=h_rep[:, :],
                        op0=ALU.mult,
                        op1=ALU.mult,
                        accum_out=lacc[:, t : t + 1],
                    ).then_inc(sL, 1)
                    vector.drain()
                # bias add, then logits transpose copy-out
                vector.tensor_tensor(
                    lbias[:, :], lacc[:, :], out_b_sb[:, :], op=ALU.add
                ).then_inc(sL, 1)
                vector.wait_ge(sP, P_LT)
                vector.tensor_copy(lfin[:, :], ps[0:VT, 0, 0:P]).then_inc(sV, 1)  # V_LFIN

    return nc


_NC_CACHE = None


def _get_nc():
    global _NC_CACHE
    if _NC_CACHE is None:
        _NC_CACHE = build_nc()
    return _NC_CACHE


def _shard_inputs(inputs):
    f32 = np.float32
    token = np.asarray(inputs["token"]).reshape(-1)
    tok = int(token[0])
    h = np.asarray(inputs["hn"], f32)[0, 0]            # [H]
    c = np.asarray(inputs["cn"], f32)[0, 0]            # [H]
    enc = np.asarray(inputs["encoder_outputs"], f32)[:, 0, :]  # [L, H]
    emb = np.asarray(inputs["emb"], f32)               # [V, E]
    w_ih = np.asarray(inputs["W_ih"], f32)             # [4H, 2E]
    w_hh = np.asarray(inputs["W_hh"], f32)             # [4H, H]
    b = (np.asarray(inputs["b_ih"], f32) + np.asarray(inputs["b_hh"], f32))  # [4H]
    attn_w = np.asarray(inputs["attn_W"], f32)         # [E, H]
    attn_b = np.asarray(inputs["attn_b"], f32)         # [E]
    out_w = np.asarray(inputs["out_W"], f32)           # [V, H]
    out_b = np.asarray(inputs["out_b"], f32)           # [V]

    h_cols = np.ascontiguousarray(h.reshape(HC, P).T)  # [P, HC]
    attn_wT = np.ascontiguousarray(attn_w.T)           # [H, E]
    attn_b_t = attn_b[None, :]                         # [1, E]
    w_comb = np.concatenate([w_ih, w_hh], axis=1)      # [4H, 2E+H] = [4H, 2H]

    emb_pad = np.zeros((ES * NCORES, E), f32)
    emb_pad[:V] = emb
    out_w_pad = np.zeros((VS * NCORES, H), f32)
    out_w_pad[:V] = out_w
    out_b_pad = np.zeros(VS * NCORES, f32)
    out_b_pad[:V] = out_b

    in_maps = []
    for k in range(NCORES):
        lsl = slice(k * LS, (k + 1) * LS)
        enc_k = enc[lsl]
        gidx = np.concatenate([g * H + k * P + np.arange(P) for g in range(4)])
        w_sh = w_comb[gidx]                            # [GS, 2H]
        off = np.uint32((tok - k * ES) % (1 << 32))
        in_maps.append({
            "encT": np.ascontiguousarray(enc_k.T),
            "enc_nat": np.ascontiguousarray(enc_k),
            "h_cols": h_cols,
            "c_sh": np.ascontiguousarray(c[k * P : (k + 1) * P][None, :]),
            "w_combT": np.ascontiguousarray(w_sh.T),
            "b_comb": np.ascontiguousarray(b[gidx][None, :]),
            "attn_wT": attn_wT,
            "attn_b_t": attn_b_t,
            "emb_sh": np.ascontiguousarray(emb_pad[k * ES : (k + 1) * ES]),
            "tok_off": np.array([[off], [off]], np.uint32),
            "out_w_nat": np.ascontiguousarray(out_w_pad[k * VS : (k + 1) * VS]),
            "out_b_cols": np.ascontiguousarray(
                out_b_pad[k * VS : (k + 1) * VS].reshape(VT, P).T
            ),
        })
    return in_maps


def run(inputs, trace=False, **kw):
    nc = _get_nc()
    in_maps = _shard_inputs(inputs)
    br = bass_utils.run_bass_kernel_spmd(
        nc, in_maps, list(range(NCORES)), trace=trace, **kw
    )
    logits = np.concatenate(
        [r["logits_sh"].reshape(-1) for r in br.results]
    )[:V][None, :].astype(np.float32)
    h_new = np.concatenate([r["h_new_sh"].reshape(-1) for r in br.results])
    c_new = np.concatenate([r["c_new_sh"].reshape(-1) for r in br.results])
    out = (
        logits,
        (
            h_new[None, None, :].astype(np.float32),
            c_new[None, None, :].astype(np.float32),
        ),
    )
    return out, br


def kernel(**inputs):
    out, _ = run(inputs)
    return out


# revision 26
# speedup vs baseline: 1.0109x; 1.0109x over previous
"""Trainium2 Bass kernel for nn_Chord_decoder (attention + LSTM cell + vocab head).

Distribution over 8 NeuronCores (tensor-parallel, per sharding hint):
  - encoder_outputs sharded over L (512 rows/core, both layouts) with a
    softmax-stats AllGather to combine attention partials,
  - W_ih/W_hh sharded over the 4H gate dim (gate-interleaved so core k
    owns h/c slice [128k:128k+128]), h_new recombined with an AllGather,
  - out_W/out_b/emb sharded over the vocab dim; the big out_W GEMV runs
    on the Vector engine (fused multiply+sum scalar_tensor_tensor)
    against natural-layout weight tiles streamed through a 41-slot ring.

The ring's upper 20 slots double as the attention/gate weight buffers:
encT/enc_nat/attn_W.T/W_comb.T are DMA'd there first, consumed by the
PE phases, then overwritten by the out_W stream once the consuming
phase's milestone retires — so nearly 21 MB of the 26 MB out_W shard
can prefetch before h_new is even known, keeping the DMA engines
saturated from t=0.

Raw-Bass SPMD program (one Block, manual semaphores): the Tile layer's
attached-wait encoding is not supported by this walrus build.

kernel(**inputs) accepts the FULL unsharded inputs and returns the full
(logits, (h_n, c_n)) structure, matching the reference.
"""

import sys
from contextlib import ExitStack

for _p in ("/root/.axon_site/_ro/trn_rl_repo", "/opt/trn_rl_repo"):
    if _p not in sys.path:
        sys.path.append(_p)

import numpy as np

import concourse.bass as bass
import concourse.mybir as mybir
from concourse.bass import ts
from concourse import bass_utils

F32 = mybir.dt.float32
U32 = mybir.dt.uint32
AF = mybir.ActivationFunctionType
ALU = mybir.AluOpType
AX = mybir.AxisListType

NCORES = 8
P = 128
H, E, V, L = 1024, 512, 50257, 4096
LS = L // NCORES          # 512  L rows per core
GS = 4 * H // NCORES      # 512  gate rows per core (4 gates x 128)
HC = H // P               # 8    128-chunks of H
VS = 6400                 # vocab rows per core (padded: 8*6400 = 51200)
VT = VS // P              # 50   vocab tiles per core
ES = -(-V // NCORES)      # 6283 emb rows per core (padded: 8*6283 = 50264)
AG1 = 2 + H + E           # [m, Z, u(1024), x_emb(512)] = 1538
RB = 41                   # ring slots (21 plain + 20 aliased over phase-A/B weights)
S_ENCT, S_ENCN, S_ATTN, S_WC = 21, 25, 29, 33   # alias slot bases

# semaphore milestone values
P_SCORES, P_PT, P_U, P_ST, P_WT, P_CTX, P_CC, P_PROJ, P_XC, P_GATES, P_LT = range(1, 12)
(V_ONES, V_NEGM, V_PCOLS, V_AG1, V_NEGMG, V_WROW, V_W8, V_CTXSB, V_CTXCOLS,
 V_XY, V_XCOLS, V_GATESSB, V_CNEW, V_HNEW, V_LFIN) = range(1, 16)
C_EXP, C_EROW, C_ACTS, C_TANH = 1, 2, 6, 7
M_AG1IN, M_AGSB, M_AG2IN, M_HREP = 16, 32, 48, 64
G_WU, G_AG1, G_AG2 = 1, 2, 3
D_SM, D_ENC, D_ENCN, D_ATTN, D_WC = 8 * 16, 8 * 16, 4 * 16, 8 * 16, 16 * 16


def build_nc():
    nc = bass.Bass(num_devices=NCORES)

    # ---- per-core inputs (host pre-sharded / pre-transposed) ----
    encT = nc.dram_tensor("encT", [H, LS], F32, kind="ExternalInput")
    enc_nat = nc.dram_tensor("enc_nat", [LS, H], F32, kind="ExternalInput")
    h_cols = nc.dram_tensor("h_cols", [P, HC], F32, kind="ExternalInput")
    c_sh = nc.dram_tensor("c_sh", [1, P], F32, kind="ExternalInput")
    w_combT = nc.dram_tensor("w_combT", [2 * H, GS], F32, kind="ExternalInput")
    b_comb = nc.dram_tensor("b_comb", [1, GS], F32, kind="ExternalInput")
    attn_wT = nc.dram_tensor("attn_wT", [H, E], F32, kind="ExternalInput")
    attn_b_t = nc.dram_tensor("attn_b_t", [1, E], F32, kind="ExternalInput")
    emb_sh = nc.dram_tensor("emb_sh", [ES, E], F32, kind="ExternalInput")
    tok_off = nc.dram_tensor("tok_off", [2, 1], U32, kind="ExternalInput")
    out_w_nat = nc.dram_tensor("out_w_nat", [VS, H], F32, kind="ExternalInput")
    out_b_cols = nc.dram_tensor("out_b_cols", [P, VT], F32, kind="ExternalInput")

    # ---- per-core outputs ----
    logits_sh = nc.dram_tensor("logits_sh", [VT, P], F32, kind="ExternalOutput")
    h_new_sh = nc.dram_tensor("h_new_sh", [1, P], F32, kind="ExternalOutput")
    c_new_sh = nc.dram_tensor("c_new_sh", [1, P], F32, kind="ExternalOutput")

    # ---- NEFF-embedded constants ----
    eye8_d = nc.inline_tensor(np.eye(8, dtype=np.float32), "eye8")
    eye128_d = nc.inline_tensor(np.eye(128, dtype=np.float32), "eye128")

    # ---- internal DRAM (collective bounce buffers) ----
    wu_in = nc.dram_tensor("wu_in", [1, 8], F32)
    wu_out = nc.dram_tensor("wu_out", [NCORES, 8], F32)
    ag1_in = nc.dram_tensor("ag1_in", [1, AG1], F32)
    ag1_out = nc.dram_tensor("ag1_out", [NCORES, AG1], F32)
    ag2_in = nc.dram_tensor("ag2_in", [1, P], F32)
    ag2_out = nc.dram_tensor("ag2_out", [NCORES, P], F32)

    groups = [list(range(NCORES))]

    with ExitStack() as st:
        sb = lambda name, shape, dt=F32: st.enter_context(
            nc.sbuf_tensor(name, shape, dt)
        )
        w_ring = sb("w_ring", [P, RB, H])
        h_rep = sb("h_rep", [P, H])
        h_cols_sb = sb("h_cols_sb", [P, HC])
        c_sb = sb("c_sb", [1, P])
        b_comb_sb = sb("b_comb_sb", [1, GS])
        attn_b_sb = sb("attn_b_sb", [1, E])
        out_b_sb = sb("out_b_sb", [P, VT])
        eye8_sb = sb("eye8_sb", [8, 8])
        eye128_sb = sb("eye128_sb", [P, P])
        tok_sb = sb("tok_sb", [2, 1], U32)
        wu_sb = sb("wu_sb", [1, 8])
        ones1 = sb("ones1", [1, 1])
        ones8 = sb("ones8", [8, 1])
        m_loc = sb("m_loc", [1, 1])
        neg_m = sb("neg_m", [1, 1])
        p_vec = sb("p_vec", [1, LS])
        z_loc = sb("z_loc", [1, 1])
        p_cols = sb("p_cols", [P, LS // P])
        xe_sb = sb("xe_sb", [2, E])
        ag1_sb = sb("ag1_sb", [1, AG1])
        ag_sb = sb("ag_sb", [NCORES, AG1])
        m_glob = sb("m_glob", [1, 1])
        neg_mg = sb("neg_mg", [1, 1])
        e_row = sb("e_row", [1, 8])
        zsc = sb("zsc", [1, 8])
        z_glob = sb("z_glob", [1, 1])
        z_inv = sb("z_inv", [1, 1])
        w_row = sb("w_row", [1, 8])
        w8_sb = sb("w8_sb", [8, 1])
        ctx_sb = sb("ctx_sb", [1, H])
        ctx_cols = sb("ctx_cols", [P, HC])
        xy_sb = sb("xy_sb", [1, 2 * E])
        x_cols = sb("x_cols", [P, HC])
        gates_sb = sb("gates_sb", [1, GS])
        i_s = sb("i_s", [1, P])
        f_s = sb("f_s", [1, P])
        g_t = sb("g_t", [1, P])
        o_s = sb("o_s", [1, P])
        fc = sb("fc", [1, P])
        ig = sb("ig", [1, P])
        c_new = sb("c_new", [1, P])
        tc_sb = sb("tc_sb", [1, P])
        h_new = sb("h_new", [1, P])
        lacc = sb("lacc", [P, VT])
        lbias = sb("lbias", [P, VT])
        lfin = sb("lfin", [VT, P])

        ps = st.enter_context(nc.psum_tensor("ps", [P, 8, 512], F32))

        sem = lambda name: st.enter_context(nc.semaphore(name))
        dS = sem("dS")        # small setup DMAs
        dEnc = sem("dEnc")    # encT chunk DMAs
        dEncN = sem("dEncN")  # enc_nat chunk DMAs
        dAttn = sem("dAttn")  # attn_wT chunk DMAs
        dWc = sem("dWc")      # w_combT chunk DMAs
        sP = sem("sP")        # PE milestones
        sV = sem("sV")        # DVE milestones
        sC = sem("sC")        # ACT milestones
        sM = sem("sM")        # mid-phase DMAs (ACT-issued)
        sG = sem("sG")        # collectives
        sX = sem("sX")        # emb gather DMA
        sXm = sem("sXm")      # gpsimd memset ordering
        sWu = sem("sWu")      # warmup buffer DMA
        sL = sem("sL")        # per-STT logits progress
        sO = sem("sO")        # output DMAs
        wsem = [sem(f"wsl{j}") for j in range(RB)]   # per-ring-slot DMA sems

        # phase-A/B views into the aliased ring slots
        def enc_v(c):       # [128, 512] h-chunk c of encT
            return w_ring[:, S_ENCT + c // 2, (c % 2) * 512 : (c % 2) * 512 + 512]

        def encn_v(c, nb):  # [128, 512] l-chunk c, h-half nb of enc_nat
            return w_ring[:, S_ENCN + c, nb * 512 : nb * 512 + 512]

        def attn_v(c):      # [128, 512] h-chunk c of attn_wT
            return w_ring[:, S_ATTN + c // 2, (c % 2) * 512 : (c % 2) * 512 + 512]

        def wc_v(c):        # [128, 512] x-chunk c of w_combT
            return w_ring[:, S_WC + c // 2, (c % 2) * 512 : (c % 2) * 512 + 512]

        with nc.Block() as block:

            @block.sync
            def _(sync):
                # small setup DMAs first (8 x then_inc(dS, 16))
                sync.dma_start(out=h_cols_sb[:, :], in_=h_cols[:, :]).then_inc(dS, 16)
                sync.dma_start(out=c_sb[:, :], in_=c_sh[:, :]).then_inc(dS, 16)
                sync.dma_start(out=b_comb_sb[:, :], in_=b_comb[:, :]).then_inc(dS, 16)
                sync.dma_start(out=attn_b_sb[:, :], in_=attn_b_t[:, :]).then_inc(dS, 16)
                sync.dma_start(out=out_b_sb[:, :], in_=out_b_cols[:, :]).then_inc(dS, 16)
                sync.dma_start(out=eye8_sb[:, :], in_=eye8_d[:, :]).then_inc(dS, 16)
                sync.dma_start(out=eye128_sb[:, :], in_=eye128_d[:, :]).then_inc(dS, 16)
                sync.dma_start(out=tok_sb[:, :], in_=tok_off[:, :]).then_inc(dS, 16)
                # phase-A/B weights into the aliased upper ring slots
                for c in range(HC):
                    sync.dma_start(
                        out=enc_v(c), in_=encT[ts(c, P), :]
                    ).then_inc(dEnc, 16)
                for c in range(LS // P):
                    sync.dma_start(
                        out=w_ring[:, S_ENCN + c, :], in_=enc_nat[ts(c, P), :]
                    ).then_inc(dEncN, 16)
                for c in range(HC):
                    sync.dma_start(
                        out=attn_v(c), in_=attn_wT[ts(c, P), :]
                    ).then_inc(dAttn, 16)
                for c in range(2 * HC):
                    sync.dma_start(
                        out=wc_v(c), in_=w_combT[ts(c, P), :]
                    ).then_inc(dWc, 16)
                # out_W stream through the ring
                for t in range(VT):
                    j = t % RB
                    if t == S_ENCT:
                        sync.wait_ge(sP, P_SCORES)
                    elif t == S_ENCN:
                        sync.wait_ge(sP, P_U)
                    elif t == S_ATTN:
                        sync.wait_ge(sP, P_PROJ)
                    elif t == S_WC:
                        sync.wait_ge(sP, P_GATES)
                    elif t >= RB:
                        sync.wait_ge(sL, t - RB + 1)
                    sync.dma_start(
                        out=w_ring[:, j, :], in_=out_w_nat[ts(t, P), :]
                    ).then_inc(wsem[j], 16)
                # final logits store
                sync.wait_ge(sV, V_LFIN)
                sync.dma_start(out=logits_sh[:, :], in_=lfin[:, :]).then_inc(sO, 16)
                sync.wait_ge(sO, 48)

            @block.gpsimd
            def _(gpsimd):
                # warmup collective: absorbs cross-core start skew + cc init
                gpsimd.memset(wu_sb[:, :], 0.0).then_inc(sXm, 1)
                gpsimd.wait_ge(sXm, 1)
                gpsimd.dma_start(out=wu_in[:, :], in_=wu_sb[:, :]).then_inc(sWu, 16)
                gpsimd.wait_ge(sWu, 16)
                gpsimd.collective_compute(
                    "AllGather",
                    ALU.bypass,
                    replica_groups=groups,
                    ins=[wu_in[:, :].opt()],
                    outs=[wu_out[:, :].opt()],
                ).then_inc(sG, 1)
                gpsimd.memset(xe_sb[:, :], 0.0).then_inc(sXm, 1)
                gpsimd.wait_ge(dS, D_SM)
                gpsimd.wait_ge(sXm, 2)
                gpsimd.indirect_dma_start(
                    out=xe_sb[:, :],
                    out_offset=None,
                    in_=emb_sh[:, :],
                    in_offset=bass.IndirectOffsetOnAxis(ap=tok_sb[:, :1], axis=0),
                    bounds_check=ES - 1,
                    oob_is_err=False,
                ).then_inc(sX, 16)
                gpsimd.wait_ge(sM, M_AG1IN)
                gpsimd.collective_compute(
                    "AllGather",
                    ALU.bypass,
                    replica_groups=groups,
                    ins=[ag1_in[:, :].opt()],
                    outs=[ag1_out[:, :].opt()],
                ).then_inc(sG, 1)
                gpsimd.wait_ge(sM, M_AG2IN)
                gpsimd.collective_compute(
                    "AllGather",
                    ALU.bypass,
                    replica_groups=groups,
                    ins=[ag2_in[:, :].opt()],
                    outs=[ag2_out[:, :].opt()],
                ).then_inc(sG, 1)

            @block.tensor
            def _(tensor):
                tensor.wait_ge(dS, D_SM)
                tensor.wait_ge(dEnc, D_ENC)
                # scores = enc @ h  -> ps[b0] [1, 512]
                for c in range(HC):
                    mm = tensor.matmul(
                        ps[0:1, 0, :],
                        lhsT=h_cols_sb[:, c : c + 1],
                        rhs=enc_v(c),
                        start=(c == 0),
                        stop=(c == HC - 1),
                    )
                mm.then_inc(sP, 1)  # P_SCORES
                # p transposes -> ps[b1] [128, 4]
                tensor.wait_ge(sC, C_EXP)
                tensor.wait_ge(sV, V_ONES)
                for j in range(LS // P):
                    mm = tensor.matmul(
                        ps[0:P, 1, j : j + 1],
                        lhsT=p_vec[0:1, ts(j, P)],
                        rhs=ones1[0:1, 0:1],
                        start=True,
                        stop=True,
                    )
                mm.then_inc(sP, 1)  # P_PT
                # u = p @ enc -> ps[b2:b4] [1, 1024]
                tensor.wait_ge(sV, V_PCOLS)
                tensor.wait_ge(dEncN, D_ENCN)
                for c in range(LS // P):
                    for nb in range(2):
                        mm = tensor.matmul(
                            ps[0:1, 2 + nb, :],
                            lhsT=p_cols[:, c : c + 1],
                            rhs=encn_v(c, nb),
                            start=(c == 0),
                            stop=(c == LS // P - 1),
                        )
                mm.then_inc(sP, 1)  # P_U
                # gathered stats transposes -> ps[b4] [1, 16]
                tensor.wait_ge(sM, M_AGSB)
                tensor.matmul(
                    ps[0:1, 4, 0:8], lhsT=ag_sb[:, 0:1], rhs=eye8_sb[:, :],
                    start=True, stop=True,
                )
                tensor.matmul(
                    ps[0:1, 4, 8:16], lhsT=ag_sb[:, 1:2], rhs=eye8_sb[:, :],
                    start=True, stop=True,
                ).then_inc(sP, 1)  # P_ST
                # w transpose -> ps[b5] [8, 1]
                tensor.wait_ge(sV, V_WROW)
                tensor.matmul(
                    ps[0:8, 5, 0:1], lhsT=w_row[0:1, :], rhs=ones1[0:1, 0:1],
                    start=True, stop=True,
                ).then_inc(sP, 1)  # P_WT
                # ctx combine -> ps[b0:b2] [1, 1024]; x_emb sum -> ps[b6] [1, 512]
                tensor.wait_ge(sV, V_W8)
                for nb in range(2):
                    tensor.matmul(
                        ps[0:1, nb, :],
                        lhsT=w8_sb[:, 0:1],
                        rhs=ag_sb[:, 2 + nb * 512 : 2 + (nb + 1) * 512],
                        start=True,
                        stop=True,
                    )
                tensor.matmul(
                    ps[0:1, 6, :], lhsT=ones8[:, 0:1], rhs=ag_sb[:, 2 + H : AG1],
                    start=True, stop=True,
                ).then_inc(sP, 1)  # P_CTX
                # ctx transposes -> ps[b7] [128, 8]
                tensor.wait_ge(sV, V_CTXSB)
                for j in range(HC):
                    mm = tensor.matmul(
                        ps[0:P, 7, j : j + 1],
                        lhsT=ctx_sb[0:1, ts(j, P)],
                        rhs=ones1[0:1, 0:1],
                        start=True,
                        stop=True,
                    )
                mm.then_inc(sP, 1)  # P_CC
                # attention projection -> ps[b4] [1, 512]
                tensor.wait_ge(sV, V_CTXCOLS)
                tensor.wait_ge(dAttn, D_ATTN)
                for c in range(HC):
                    mm = tensor.matmul(
                        ps[0:1, 4, :],
                        lhsT=ctx_cols[:, c : c + 1],
                        rhs=attn_v(c),
                        start=(c == 0),
                        stop=(c == HC - 1),
                    )
                mm.then_inc(sP, 1)  # P_PROJ
                # x transposes -> ps[b5] [128, 8]
                tensor.wait_ge(sV, V_XY)
                for j in range(HC):
                    mm = tensor.matmul(
                        ps[0:P, 5, j : j + 1],
                        lhsT=xy_sb[0:1, ts(j, P)],
                        rhs=ones1[0:1, 0:1],
                        start=True,
                        stop=True,
                    )
                mm.then_inc(sP, 1)  # P_XC
                # LSTM gates -> ps[b6] [1, 512]
                tensor.wait_ge(sV, V_XCOLS)
                tensor.wait_ge(dWc, D_WC)
                for c in range(2 * HC):
                    lhs = (
                        x_cols[:, c : c + 1]
                        if c < HC
                        else h_cols_sb[:, c - HC : c - HC + 1]
                    )
                    mm = tensor.matmul(
                        ps[0:1, 6, :],
                        lhsT=lhs,
                        rhs=wc_v(c),
                        start=(c == 0),
                        stop=(c == 2 * HC - 1),
                    )
                mm.then_inc(sP, 1)  # P_GATES
                # final logits transpose -> ps[b0] [50, 128]
                tensor.wait_ge(sL, VT + 1)
                tensor.transpose(
                    ps[0:VT, 0, 0:P], lbias[:, :], eye128_sb[:, :]
                ).then_inc(sP, 1)  # P_LT

            @block.scalar
            def _(scalar):
                # exp(scores - m) with fused sum
                scalar.wait_ge(sP, P_SCORES)
                scalar.wait_ge(sV, V_NEGM)
                scalar.activation(
                    p_vec[:, :], ps[0:1, 0, :], AF.Exp, bias=neg_m[0:1, :],
                    scale=1.0, accum_out=z_loc[:, :],
                ).then_inc(sC, 1)  # C_EXP
                # mid DMAs: ag1 out/in
                scalar.wait_ge(sV, V_AG1)
                scalar.dma_start(out=ag1_in[:, :], in_=ag1_sb[:, :]).then_inc(sM, 16)
                scalar.wait_ge(sG, G_AG1)
                scalar.dma_start(out=ag_sb[:, :], in_=ag1_out[:, :]).then_inc(sM, 16)
                # e_row = exp(m_p - M)
                scalar.wait_ge(sP, P_ST)
                scalar.wait_ge(sV, V_NEGMG)
                scalar.activation(
                    e_row[:, :], ps[0:1, 4, 0:8], AF.Exp, bias=neg_mg[0:1, :]
                ).then_inc(sC, 1)  # C_EROW
                # gate activations
                scalar.wait_ge(sV, V_GATESSB)
                scalar.activation(i_s[:, :], gates_sb[0:1, 0:P], AF.Sigmoid).then_inc(sC, 1)
                scalar.activation(f_s[:, :], gates_sb[0:1, P : 2 * P], AF.Sigmoid).then_inc(sC, 1)
                scalar.activation(g_t[:, :], gates_sb[0:1, 2 * P : 3 * P], AF.Tanh).then_inc(sC, 1)
                scalar.activation(o_s[:, :], gates_sb[0:1, 3 * P : 4 * P], AF.Sigmoid).then_inc(sC, 1)  # C_ACTS
                scalar.wait_ge(sV, V_CNEW)
                scalar.activation(tc_sb[:, :], c_new[0:1, :], AF.Tanh).then_inc(sC, 1)  # C_TANH
                # outputs + AG2 feed + h replicate
                scalar.wait_ge(sV, V_HNEW)
                scalar.dma_start(out=h_new_sh[:, :], in_=h_new[:, :]).then_inc(sO, 16)
                scalar.dma_start(out=c_new_sh[:, :], in_=c_new[:, :]).then_inc(sO, 16)
                scalar.dma_start(out=ag2_in[:, :], in_=h_new[:, :]).then_inc(sM, 16)
                scalar.wait_ge(sG, G_AG2)
                scalar.dma_start(
                    out=h_rep[:, :],
                    in_=ag2_out[:, :].rearrange("a b -> (a b)").partition_broadcast(P),
                ).then_inc(sM, 16)

            @block.vector
            def _(vector):
                vector.memset(ones1[:, :], 1.0)
                vector.memset(ones8[:, :], 1.0).then_inc(sV, 1)  # V_ONES
                # local softmax stats
                vector.wait_ge(sP, P_SCORES)
                vector.reduce_max(m_loc[:, :], ps[0:1, 0, :], axis=AX.X)
                vector.drain()
                vector.tensor_scalar_mul(neg_m[:, :], m_loc[:, :], -1.0).then_inc(sV, 1)  # V_NEGM
                vector.wait_ge(sP, P_PT)
                vector.tensor_copy(p_cols[:, :], ps[0:P, 1, 0:4]).then_inc(sV, 1)  # V_PCOLS
                # assemble AG1 payload
                vector.wait_ge(sP, P_U)
                vector.wait_ge(sC, C_EXP)
                vector.wait_ge(sX, 16)
                vector.tensor_copy(ag1_sb[0:1, 0:1], m_loc[:, :])
                vector.tensor_copy(ag1_sb[0:1, 1:2], z_loc[:, :])
                vector.tensor_copy(ag1_sb[0:1, 2 : 2 + 512], ps[0:1, 2, :])
                vector.tensor_copy(ag1_sb[0:1, 2 + 512 : 2 + H], ps[0:1, 3, :])
                vector.tensor_copy(ag1_sb[0:1, 2 + H : AG1], xe_sb[0:1, :]).then_inc(sV, 1)  # V_AG1
                # global softmax combine
                vector.wait_ge(sP, P_ST)
                vector.reduce_max(m_glob[:, :], ps[0:1, 4, 0:8], axis=AX.X)
                vector.drain()
                vector.tensor_scalar_mul(neg_mg[:, :], m_glob[:, :], -1.0).then_inc(sV, 1)  # V_NEGMG
                vector.wait_ge(sC, C_EROW)
                vector.tensor_tensor(zsc[:, :], e_row[:, :], ps[0:1, 4, 8:16], op=ALU.mult)
                vector.drain()
                vector.reduce_sum(z_glob[:, :], zsc[:, :], axis=AX.X)
                vector.drain()
                vector.reciprocal(z_inv[:, :], z_glob[:, :])
                vector.drain()
                vector.tensor_scalar_mul(w_row[:, :], e_row[:, :], z_inv[0:1, :]).then_inc(sV, 1)  # V_WROW
                vector.wait_ge(sP, P_WT)
                vector.tensor_copy(w8_sb[:, :], ps[0:8, 5, 0:1]).then_inc(sV, 1)  # V_W8
                vector.wait_ge(sP, P_CTX)
                vector.tensor_copy(ctx_sb[0:1, 0:512], ps[0:1, 0, :])
                vector.tensor_copy(ctx_sb[0:1, 512:H], ps[0:1, 1, :]).then_inc(sV, 1)  # V_CTXSB
                vector.wait_ge(sP, P_CC)
                vector.tensor_copy(ctx_cols[:, :], ps[0:P, 7, 0:8]).then_inc(sV, 1)  # V_CTXCOLS
                vector.wait_ge(sP, P_PROJ)
                vector.tensor_add(xy_sb[0:1, 0:E], ps[0:1, 4, :], attn_b_sb[:, :])
                vector.tensor_copy(xy_sb[0:1, E : 2 * E], ps[0:1, 6, :]).then_inc(sV, 1)  # V_XY
                vector.wait_ge(sP, P_XC)
                vector.tensor_copy(x_cols[:, :], ps[0:P, 5, 0:8]).then_inc(sV, 1)  # V_XCOLS
                vector.wait_ge(sP, P_GATES)
                vector.tensor_add(gates_sb[:, :], ps[0:1, 6, :], b_comb_sb[:, :]).then_inc(sV, 1)  # V_GATESSB
                # LSTM cell pointwise
                vector.wait_ge(sC, C_ACTS)
                vector.tensor_tensor(fc[:, :], f_s[:, :], c_sb[:, :], op=ALU.mult)
                vector.tensor_tensor(ig[:, :], i_s[:, :], g_t[:, :], op=ALU.mult)
                vector.drain()
                vector.tensor_tensor(c_new[:, :], fc[:, :], ig[:, :], op=ALU.add).then_inc(sV, 1)  # V_CNEW
                vector.wait_ge(sC, C_TANH)
                vector.tensor_tensor(h_new[:, :], o_s[:, :], tc_sb[:, :], op=ALU.mult).then_inc(sV, 1)  # V_HNEW
                # vocab head: fused multiply+sum per 128-row out_W tile
                vector.wait_ge(sM, M_HREP)
                for t in range(VT):
                    j = t % RB
                    vector.wait_ge(wsem[j], 16 * (t // RB + 1))
                    # scratch product lands in retired PSUM banks (b2-3/b4-5)
                    pb = 2 + (t % 2) * 2
                    vector.scalar_tensor_tensor(
                        out=ps[0:P, pb : pb + 2, :].rearrange("p a b -> p (a b)"),
                        in0=w_ring[:, j, :],
                        scalar=1.0,
                        in1=h_rep[:, :],
                        op0=ALU.mult,
                        op1=ALU.mult,
                        accum_out=lacc[:, t : t + 1],
                    ).then_inc(sL, 1)
                    vector.drain()
                # bias add, then logits transpose copy-out
                vector.tensor_tensor(
                    lbias[:, :], lacc[:, :], out_b_sb[:, :], op=ALU.add
                ).then_inc(sL, 1)
                vector.wait_ge(sP, P_LT)
                vector.tensor_copy(lfin[:, :], ps[0:VT, 0, 0:P]).then_inc(sV, 1)  # V_LFIN

    return nc


_NC_CACHE = None


def _get_nc():
    global _NC_CACHE
    if _NC_CACHE is None:
        _NC_CACHE = build_nc()
    return _NC_CACHE


def _shard_inputs(inputs):
    f32 = np.float32
    token = np.asarray(inputs["token"]).reshape(-1)
    tok = int(token[0])
    h = np.asarray(inputs["hn"], f32)[0, 0]            # [H]
    c = np.asarray(inputs["cn"], f32)[0, 0]            # [H]
    enc = np.asarray(inputs["encoder_outputs"], f32)[:, 0, :]  # [L, H]
    emb = np.asarray(inputs["emb"], f32)               # [V, E]
    w_ih = np.asarray(inputs["W_ih"], f32)             # [4H, 2E]
    w_hh = np.asarray(inputs["W_hh"], f32)             # [4H, H]
    b = (np.asarray(inputs["b_ih"], f32) + np.asarray(inputs["b_hh"], f32))  # [4H]
    attn_w = np.asarray(inputs["attn_W"], f32)         # [E, H]
    attn_b = np.asarray(inputs["attn_b"], f32)         # [E]
    out_w = np.asarray(inputs["out_W"], f32)           # [V, H]
    out_b = np.asarray(inputs["out_b"], f32)           # [V]

    h_cols = np.ascontiguousarray(h.reshape(HC, P).T)  # [P, HC]
    attn_wT = np.ascontiguousarray(attn_w.T)           # [H, E]
    attn_b_t = attn_b[None, :]                         # [1, E]
    w_comb = np.concatenate([w_ih, w_hh], axis=1)      # [4H, 2E+H] = [4H, 2H]

    emb_pad = np.zeros((ES * NCORES, E), f32)
    emb_pad[:V] = emb
    out_w_pad = np.zeros((VS * NCORES, H), f32)
    out_w_pad[:V] = out_w
    out_b_pad = np.zeros(VS * NCORES, f32)
    out_b_pad[:V] = out_b

    in_maps = []
    for k in range(NCORES):
        lsl = slice(k * LS, (k + 1) * LS)
        enc_k = enc[lsl]
        gidx = np.concatenate([g * H + k * P + np.arange(P) for g in range(4)])
        w_sh = w_comb[gidx]                            # [GS, 2H]
        off = np.uint32((tok - k * ES) % (1 << 32))
        in_maps.append({
            "encT": np.ascontiguousarray(enc_k.T),
            "enc_nat": np.ascontiguousarray(enc_k),
            "h_cols": h_cols,
            "c_sh": np.ascontiguousarray(c[k * P : (k + 1) * P][None, :]),
            "w_combT": np.ascontiguousarray(w_sh.T),
            "b_comb": np.ascontiguousarray(b[gidx][None, :]),
            "attn_wT": attn_wT,
            "attn_b_t": attn_b_t,
            "emb_sh": np.ascontiguousarray(emb_pad[k * ES : (k + 1) * ES]),
            "tok_off": np.array([[off], [off]], np.uint32),
            "out_w_nat": np.ascontiguousarray(out_w_pad[k * VS : (k + 1) * VS]),
            "out_b_cols": np.ascontiguousarray(
                out_b_pad[k * VS : (k + 1) * VS].reshape(VT, P).T
            ),
        })
    return in_maps


def run(inputs, trace=False, **kw):
    nc = _get_nc()
    in_maps = _shard_inputs(inputs)
    br = bass_utils.run_bass_kernel_spmd(
        nc, in_maps, list(range(NCORES)), trace=trace, **kw
    )
    logits = np.concatenate(
        [r["logits_sh"].reshape(-1) for r in br.results]
    )[:V][None, :].astype(np.float32)
    h_new = np.concatenate([r["h_new_sh"].reshape(-1) for r in br.results])
    c_new = np.concatenate([r["c_new_sh"].reshape(-1) for r in br.results])
    out = (
        logits,
        (
            h_new[None, None, :].astype(np.float32),
            c_new[None, None, :].astype(np.float32),
        ),
    )
    return out, br


def kernel(**inputs):
    out, _ = run(inputs)
    return out
